# revision 33
# baseline (speedup 1.0000x reference)
"""MiniMind MoE FFN on 8 Trainium2 NeuronCores (Bass/Tile, SPMD).

Sharding: expert-parallel. Core e owns routed expert e (Wg/Wu/Wd[e]); tokens
routed to e are gathered host-side (dispatch) and sent as that core's
activation shard, transposed to [H, C] for the matmul layout. The shared
expert is data-parallel over tokens (T/8 per core). Combine (scatter-add of
weighted expert outputs + shared partials) happens at unshard time.

Device math per core, matmul operands in float16 (1 cycle/row on the PE,
half the HBM traffic of fp32; fp32 PSUM accumulation keeps rel-err ~5e-4):
  phase 1:  GT[i] = Wg^T x^T, UT[i] = Wu^T x^T   (I on partitions)
            AT[i] = silu(GT[i]) * UT[i]
  phase 2:  YT[h] = sum_i Wd[i][:,h].T @ AT[i]   (transposed: H on
            partitions, tokens on the free axis, so cost scales with the
            true capacity C rather than ceil(C/128) PSUM sweeps)
            yr    = YT * w[broadcast]            (top-k combine weight)
plus the same for the shared expert on this core's token slice (no w,
token-major output). No PE warmup is needed: the framework preamble's
per-engine TENSOR_LOADs already lift the HAM clock gate.
"""

import sys

sys.path.insert(0, "/opt/trn_rl_repo")

import numpy as np

import concourse.bacc as bacc
import concourse.mybir as mybir
import concourse.tile as tile

H = 512
I = 1408
E = 8
K = 2
ALPHA = 0.1
B, S = 2, 1024
T = B * S
N_CORES = 8
P = 128
HK = H // P     # 4 contraction chunks over H
IT = I // P     # 11 chunks over I
CS = T // N_CORES  # shared-expert tokens per core

F32 = mybir.dt.float32
F32R = mybir.dt.float32r

# matmul-operand dtype: float16 halves DMA vs f32 and runs 1 cycle/row on PE
# (vs ~2cyc/row + 4x weight-load cost for fp32); 10 mantissa bits keep the
# end-to-end error ~1e-3 for these O(1) activations.
import os as _os

_DT_NAME = _os.environ.get("KDT", "f16")
MM_DT = {"f32r": F32R, "f16": mybir.dt.float16, "bf16": mybir.dt.bfloat16}[_DT_NAME]


def _to_mm_np(a):
    import ml_dtypes

    np_dt = {"f32r": np.float32, "f16": np.float16, "bf16": ml_dtypes.bfloat16}[
        _DT_NAME
    ]
    return np.ascontiguousarray(np.asarray(a, np.float32).astype(np_dt))


def _install_axon_ntff_shim():
    """Provide antenv.axon_hooks if the image lacks it, so that
    run_bass_kernel_spmd(trace=True)/BASS_TRACE doesn't crash under axon."""
    try:
        import antenv.axon_hooks  # noqa: F401

        return
    except ImportError:
        pass
    import contextlib
    import ctypes
    import types

    def _make_hook():
        try:
            lib = ctypes.CDLL("/opt/axon/libaxon_pjrt.so")
            if not hasattr(lib, "axon_start_nrt_profile"):
                return None
            lib.axon_start_nrt_profile.argtypes = [
                ctypes.POINTER(ctypes.c_int64),
                ctypes.c_size_t,
            ]
            lib.axon_start_nrt_profile.restype = ctypes.c_int64
            lib.axon_stop_nrt_profile.argtypes = [ctypes.c_char_p]
            lib.axon_stop_nrt_profile.restype = ctypes.c_int64
        except OSError:
            return None

        @contextlib.contextmanager
        def _hook(output_dir, device_ids):
            import jax

            jax.devices()
            if device_ids:
                ids = (ctypes.c_int64 * len(device_ids))(*device_ids)
                rc = lib.axon_start_nrt_profile(ids, len(device_ids))
            else:
                rc = lib.axon_start_nrt_profile(None, 0)
            if rc != 0:
                raise RuntimeError(f"axon_start_nrt_profile rc={rc}")
            try:
                yield
            finally:
                n = lib.axon_stop_nrt_profile(str(output_dir).encode())
                print(f"ntff profile: {n} file(s) in {output_dir}", file=sys.stderr)

        return _hook

    hook = _make_hook()
    mod = types.ModuleType("antenv.axon_hooks")
    mod.get_axon_ntff_profile_hook = lambda: hook
    mod.set_axon_ntff_profile_hook = lambda h: None
    sys.modules["antenv.axon_hooks"] = mod


def _blocks(total, width):
    return [(b0, min(width, total - b0)) for b0 in range(0, total, width)]


def _chunks(total, maxc):
    n = -(-total // maxc)
    base, rem = divmod(total, n)
    out = []
    off = 0
    for j in range(n):
        ck = base + (1 if j < rem else 0)
        out.append((off, ck))
        off += ck
    return out


def build(C):
    """Build the SPMD single-core program for routed capacity C (mult of 64)."""
    nc = bacc.Bacc("TRN2", target_bir_lowering=False, debug=False, num_devices=N_CORES)

    TOKT = -(-C // P)          # routed token tiles (last may be partial)
    Cp = TOKT * P              # w vector padded to full tiles
    ttiles = [(t * P, min(P, C - t * P)) for t in range(TOKT)]
    tokt_s = CS // P
    cchunks = _chunks(C, 512)

    xg_d = nc.dram_tensor("xg", [H, C], MM_DT, kind="ExternalInput").ap()
    w_d = nc.dram_tensor("w", [P, C], F32, kind="ExternalInput").ap()
    wg_d = nc.dram_tensor("wg", [H, I], MM_DT, kind="ExternalInput").ap()
    wu_d = nc.dram_tensor("wu", [H, I], MM_DT, kind="ExternalInput").ap()
    wd_d = nc.dram_tensor("wd", [I, H], MM_DT, kind="ExternalInput").ap()
    xs_d = nc.dram_tensor("xs", [H, CS], MM_DT, kind="ExternalInput").ap()
    wgs_d = nc.dram_tensor("wgs", [H, I], MM_DT, kind="ExternalInput").ap()
    wus_d = nc.dram_tensor("wus", [H, I], MM_DT, kind="ExternalInput").ap()
    wds_d = nc.dram_tensor("wds", [I, H], MM_DT, kind="ExternalInput").ap()
    yr_d = nc.dram_tensor("yr", [H, C], F32, kind="ExternalOutput").ap()
    ys_d = nc.dram_tensor("ys", [CS, H], F32, kind="ExternalOutput").ap()

    with tile.TileContext(nc) as tc:
        with (
            tc.tile_pool(name="wbig", bufs=4 * HK) as wbig,
            tc.tile_pool(name="wshp", bufs=2 * HK) as wshp,
            tc.tile_pool(name="wdp", bufs=IT) as wdp,
            tc.tile_pool(name="wdsp", bufs=IT) as wdsp,
            tc.tile_pool(name="xp", bufs=HK) as xp,
            tc.tile_pool(name="xsp", bufs=HK) as xsp,
            tc.tile_pool(name="atp", bufs=IT) as atp,
            tc.tile_pool(name="atsp", bufs=IT) as atsp,
            tc.tile_pool(name="gtp", bufs=3) as gtp,
            tc.tile_pool(name="op", bufs=4) as op,
            tc.tile_pool(name="wsp", bufs=1) as wsp,
            tc.tile_pool(name="gps", bufs=4, space="PSUM") as gps,
            tc.tile_pool(name="yps", bufs=4, space="PSUM") as yps,
        ):
            nm = iter(range(10**6))

            # ---- loads for the routed expert (in first-needed order) ----
            xg_sb = []
            for k in range(HK):
                t = xp.tile([P, C], MM_DT, tag="xg", name=f"xg{k}")
                nc.scalar.dma_start(t[:], xg_d[k * P:(k + 1) * P, :])
                xg_sb.append(t)

            # wg/wu streamed in 2 column halves, dual-queue issue: wg on the
            # sync queue, wu on the scalar queue (both HWDGE-capable) so the
            # ~600ns/DMA serial issue cost is paid in parallel. The scalar
            # queue gets ONLY these 8 early DMAs - its silu work starts
            # after they are issued, so nothing blocks.
            wblocks = [(0, 768), (768, I - 768)]
            wg_sb = [[None] * len(wblocks) for _ in range(HK)]
            wu_sb = [[None] * len(wblocks) for _ in range(HK)]
            for b, (b0, bw) in enumerate(wblocks):
                for lst, src, eng in (
                    (wg_sb, wg_d, nc.sync),
                    (wu_sb, wu_d, nc.sync),
                ):
                    for k in range(HK):
                        t = wbig.tile([P, bw], MM_DT, tag="wb", name=f"wb{next(nm)}")
                        eng.dma_start(t[:], src[k * P:(k + 1) * P, b0:b0 + bw])
                        lst[k][b] = t

            def wslice(lst, k, i):
                c = i * P
                b = 0 if c < wblocks[0][1] else 1
                off = c - wblocks[b][0]
                return lst[k][b][:, off:off + P]

            wd_sb = []
            for i in range(IT):
                t = wdp.tile([P, H], MM_DT, tag="wd", name=f"wd{i}")
                nc.sync.dma_start(t[:], wd_d[i * P:(i + 1) * P, :])
                wd_sb.append(t)

            # w is not needed until routed phase 2 - load late so it does
            # not steal early HBM bandwidth from the phase-1 weight stream
            w_sb = wsp.tile([P, C], F32, tag="w")
            nc.sync.dma_start(w_sb[:], w_d[:])

            # ---- loads for the shared expert (scalar queue) ----
            xs_sb = []
            for k in range(HK):
                t = xsp.tile([P, CS], MM_DT, tag="xs", name=f"xs{k}")
                nc.sync.dma_start(t[:], xs_d[k * P:(k + 1) * P, :])
                xs_sb.append(t)
            wgs_sb, wus_sb = [], []
            for wlist, src in ((wgs_sb, wgs_d), (wus_sb, wus_d)):
                for k in range(HK):
                    t = wshp.tile([P, I], MM_DT, tag="wsh", name=f"wb{next(nm)}")
                    nc.sync.dma_start(t[:], src[k * P:(k + 1) * P, :])
                    wlist.append(t)
            wds_sb = []
            for i in range(IT):
                t = wdsp.tile([P, H], MM_DT, tag="wds", name=f"wds{i}")
                nc.sync.dma_start(t[:], wds_d[i * P:(i + 1) * P, :])
                wds_sb.append(t)

            # ---- routed phase 1: AT[i] = silu(Wg^T x) * (Wu^T x) ----
            at_sb = []
            for i in range(IT):
                at_i = atp.tile([P, C], MM_DT, tag="at", name=f"at{i}")
                for (c0, ck) in cchunks:
                    pg = gps.tile([P, ck], F32, tag="pg", name=f"pg{next(nm)}")
                    pu = gps.tile([P, ck], F32, tag="pg", name=f"pu{next(nm)}")
                    for k in range(HK):
                        nc.tensor.matmul(
                            pg[:], wslice(wg_sb, k, i), xg_sb[k][:, c0:c0 + ck],
                            start=(k == 0), stop=(k == HK - 1),
                        )
                    for k in range(HK):
                        nc.tensor.matmul(
                            pu[:], wslice(wu_sb, k, i), xg_sb[k][:, c0:c0 + ck],
                            start=(k == 0), stop=(k == HK - 1),
                        )
                    gt = gtp.tile([P, ck], F32, tag="gt", name=f"gt{next(nm)}")
                    nc.scalar.activation(
                        gt[:], pg[:], mybir.ActivationFunctionType.Silu
                    )
                    nc.vector.tensor_mul(at_i[:, c0:c0 + ck], gt[:], pu[:])
                at_sb.append(at_i)

            # ---- routed phase 2 (transposed): YT[h,:] = sum_i Wd[i][:,h].T @ AT[i],
            # then scale along the token (free) axis by w. Cost scales with C
            # itself rather than ceil(C/128) PSUM sweeps of 512 rows. ----
            for hpair in range(0, HK, 2):
                pyt = {}
                for h in range(hpair, hpair + 2):
                    for ci, (c0, ck) in enumerate(cchunks):
                        pyt[h, ci] = yps.tile([P, max(c[1] for c in cchunks)], F32,
                                              tag="py", name=f"pyt{h}_{ci}")
                for i in range(IT):
                    for h in range(hpair, hpair + 2):
                        for ci, (c0, ck) in enumerate(cchunks):
                            nc.tensor.matmul(
                                pyt[h, ci][:, :ck],
                                wd_sb[i][:, h * P:(h + 1) * P],
                                at_sb[i][:, c0:c0 + ck],
                                start=(i == 0), stop=(i == IT - 1),
                            )
                for h in range(hpair, hpair + 2):
                    o = op.tile([P, C], F32, tag="ot", name=f"o{next(nm)}")
                    for ci, (c0, ck) in enumerate(cchunks):
                        nc.vector.tensor_mul(
                            o[:, c0:c0 + ck], pyt[h, ci][:, :ck],
                            w_sb[:, c0:c0 + ck],
                        )
                    nc.sync.dma_start(yr_d[h * P:(h + 1) * P, :], o[:])

            # ---- shared phase 1 ----
            ats_sb = []
            for i in range(IT):
                at_i = atsp.tile([P, CS], MM_DT, tag="ats", name=f"ats{i}")
                pg = gps.tile([P, CS], F32, tag="pg", name=f"pg{next(nm)}")
                pu = gps.tile([P, CS], F32, tag="pg", name=f"pu{next(nm)}")
                for k in range(HK):
                    nc.tensor.matmul(
                        pg[:], wgs_sb[k][:, i * P:(i + 1) * P], xs_sb[k][:],
                        start=(k == 0), stop=(k == HK - 1),
                    )
                for k in range(HK):
                    nc.tensor.matmul(
                        pu[:], wus_sb[k][:, i * P:(i + 1) * P], xs_sb[k][:],
                        start=(k == 0), stop=(k == HK - 1),
                    )
                gt = gtp.tile([P, CS], F32, tag="gt", name=f"gt{next(nm)}")
                nc.scalar.activation(gt[:], pg[:], mybir.ActivationFunctionType.Silu)
                nc.vector.tensor_mul(at_i[:], gt[:], pu[:])
                ats_sb.append(at_i)

            # ---- shared phase 2 ----
            for t in range(tokt_s):
                py = yps.tile([P, H], F32, tag="py", name=f"py{next(nm)}")
                for i in range(IT):
                    nc.tensor.matmul(
                        py[:], ats_sb[i][:, t * P:(t + 1) * P], wds_sb[i][:],
                        start=(i == 0), stop=(i == IT - 1),
                    )
                o = op.tile([P, H], F32, tag="o", name=f"o{next(nm)}")
                for half in range(2):
                    hs = slice(half * (H // 2), (half + 1) * (H // 2))
                    if half == 0:
                        nc.scalar.activation(
                            o[:, hs], py[:, hs],
                            mybir.ActivationFunctionType.Copy,
                        )
                    else:
                        nc.vector.tensor_copy(o[:, hs], py[:, hs])
                    nc.sync.dma_start(ys_d[t * P:(t + 1) * P, hs], o[:, hs])

    nc.compile()
    return nc


def _gate(x, gate_weight):
    """Host gate: softmax scores, top-2 (lax.top_k tie semantics), combine
    weights, aux loss. Mirrors the reference math in float32."""
    flat = np.ascontiguousarray(x.reshape(T, H), dtype=np.float32)
    logits = flat @ gate_weight.T.astype(np.float32)
    m = logits.max(-1, keepdims=True)
    ex = np.exp(logits - m, dtype=np.float32)
    scores = ex / ex.sum(-1, keepdims=True)
    order = np.argsort(-scores, axis=-1, kind="stable")
    topk_idx = order[:, :K]
    topk_w = np.take_along_axis(scores, topk_idx, axis=-1)
    topk_w = topk_w / (topk_w.sum(-1, keepdims=True) + np.float32(1e-20))

    onehot = np.zeros((T, K, E), np.float32)
    np.put_along_axis(onehot, topk_idx[:, :, None], np.float32(1.0), axis=2)
    counts = onehot.reshape(B, S * K, E).sum(1) / (S * K / E)
    aux = (counts * scores.reshape(B, S, E).mean(1)).sum(-1).mean() * ALPHA

    return flat, scores, topk_idx, topk_w, np.float32(aux)


def kernel(x, gate_weight, Wg, Wu, Wd, Wg_s, Wu_s, Wd_s):
    _install_axon_ntff_shim()
    x = np.asarray(x, np.float32)
    gate_weight = np.asarray(gate_weight, np.float32)
    from concourse.bass_utils import run_bass_kernel_spmd

    flat, scores, topk_idx, topk_w, aux = _gate(x, gate_weight)

    # dispatch: token lists + combine weights per expert
    idx_e, w_e = [], []
    for e in range(E):
        hit = topk_idx == e  # [T, K]
        idx = np.nonzero(hit.any(-1))[0]
        wv = (topk_w * hit).sum(-1)[idx].astype(np.float32)
        idx_e.append(idx)
        w_e.append(wv)
    C = max(512, -(-max(len(i) for i in idx_e) // 32) * 32)

    Wg = np.asarray(Wg, np.float32)
    Wu = np.asarray(Wu, np.float32)
    Wd = np.asarray(Wd, np.float32)
    in_maps = []
    for e in range(E):
        n = len(idx_e[e])
        xg = np.zeros((H, C), np.float32)
        xg[:, :n] = flat[idx_e[e]].T
        w = np.zeros((C,), np.float32)
        w[:n] = w_e[e]
        w = np.broadcast_to(w, (P, C)).copy()
        xs = flat[e * CS:(e + 1) * CS].T
        in_maps.append({
            "xg": _to_mm_np(xg),
            "w": w,
            "wg": _to_mm_np(Wg[e]),
            "wu": _to_mm_np(Wu[e]),
            "wd": _to_mm_np(Wd[e]),
            "xs": _to_mm_np(xs),
            "wgs": _to_mm_np(Wg_s),
            "wus": _to_mm_np(Wu_s),
            "wds": _to_mm_np(Wd_s),
        })

    nc = build(C)
    res = run_bass_kernel_spmd(nc, in_maps, core_ids=list(range(N_CORES)))
    kernel.last_results = res

    # combine: scatter-add routed partials, concat shared partials
    out = np.zeros((T, H), np.float32)
    for e in range(E):
        n = len(idx_e[e])
        out[idx_e[e]] += res.results[e]["yr"][:, :n].T
        out[e * CS:(e + 1) * CS] += res.results[e]["ys"]

    return out.reshape(B, S, H), aux


# revision 34
# speedup vs baseline: 1.1205x; 1.1205x over previous
"""MiniMind MoE FFN on 8 Trainium2 NeuronCores (Bass/Tile, SPMD).

Sharding: expert-parallel. Core e owns routed expert e (Wg/Wu/Wd[e]); tokens
routed to e are gathered host-side (dispatch) and sent as that core's
activation shard, transposed to [H, C] for the matmul layout. The shared
expert is data-parallel over tokens (T/8 per core). Combine (scatter-add of
weighted expert outputs + shared partials) happens at unshard time.

Device math per core, matmul operands in float16 (1 cycle/row on the PE,
half the HBM traffic of fp32; fp32 PSUM accumulation keeps rel-err ~5e-4):
  phase 1:  GT[i] = Wg^T x^T, UT[i] = Wu^T x^T   (I on partitions)
            AT[i] = silu(GT[i]) * UT[i]
  phase 2:  YT[h] = sum_i Wd[i][:,h].T @ AT[i]   (transposed: H on
            partitions, tokens on the free axis, so cost scales with the
            true capacity C rather than ceil(C/128) PSUM sweeps)
            yr    = YT * w[broadcast]            (top-k combine weight)
plus the same for the shared expert on this core's token slice (no w,
token-major output). No PE warmup is needed: the framework preamble's
per-engine TENSOR_LOADs already lift the HAM clock gate.
"""

import sys

sys.path.insert(0, "/opt/trn_rl_repo")

import numpy as np

import concourse.bacc as bacc
import concourse.mybir as mybir
import concourse.tile as tile

H = 512
I = 1408
E = 8
K = 2
ALPHA = 0.1
B, S = 2, 1024
T = B * S
N_CORES = 8
P = 128
HK = H // P     # 4 contraction chunks over H
IT = I // P     # 11 chunks over I
CS = T // N_CORES  # shared-expert tokens per core

F32 = mybir.dt.float32
F32R = mybir.dt.float32r

# matmul-operand dtype: float16 halves DMA vs f32 and runs 1 cycle/row on PE
# (vs ~2cyc/row + 4x weight-load cost for fp32); 10 mantissa bits keep the
# end-to-end error ~1e-3 for these O(1) activations.
import os as _os

_DT_NAME = _os.environ.get("KDT", "f16")
MM_DT = {"f32r": F32R, "f16": mybir.dt.float16, "bf16": mybir.dt.bfloat16}[_DT_NAME]


def _to_mm_np(a):
    import ml_dtypes

    np_dt = {"f32r": np.float32, "f16": np.float16, "bf16": ml_dtypes.bfloat16}[
        _DT_NAME
    ]
    return np.ascontiguousarray(np.asarray(a, np.float32).astype(np_dt))


def _install_axon_ntff_shim():
    """Provide antenv.axon_hooks if the image lacks it, so that
    run_bass_kernel_spmd(trace=True)/BASS_TRACE doesn't crash under axon."""
    try:
        import antenv.axon_hooks  # noqa: F401

        return
    except ImportError:
        pass
    import contextlib
    import ctypes
    import types

    def _make_hook():
        try:
            lib = ctypes.CDLL("/opt/axon/libaxon_pjrt.so")
            if not hasattr(lib, "axon_start_nrt_profile"):
                return None
            lib.axon_start_nrt_profile.argtypes = [
                ctypes.POINTER(ctypes.c_int64),
                ctypes.c_size_t,
            ]
            lib.axon_start_nrt_profile.restype = ctypes.c_int64
            lib.axon_stop_nrt_profile.argtypes = [ctypes.c_char_p]
            lib.axon_stop_nrt_profile.restype = ctypes.c_int64
        except OSError:
            return None

        @contextlib.contextmanager
        def _hook(output_dir, device_ids):
            import jax

            jax.devices()
            if device_ids:
                ids = (ctypes.c_int64 * len(device_ids))(*device_ids)
                rc = lib.axon_start_nrt_profile(ids, len(device_ids))
            else:
                rc = lib.axon_start_nrt_profile(None, 0)
            if rc != 0:
                raise RuntimeError(f"axon_start_nrt_profile rc={rc}")
            try:
                yield
            finally:
                n = lib.axon_stop_nrt_profile(str(output_dir).encode())
                print(f"ntff profile: {n} file(s) in {output_dir}", file=sys.stderr)

        return _hook

    hook = _make_hook()
    mod = types.ModuleType("antenv.axon_hooks")
    mod.get_axon_ntff_profile_hook = lambda: hook
    mod.set_axon_ntff_profile_hook = lambda h: None
    sys.modules["antenv.axon_hooks"] = mod


def _blocks(total, width):
    return [(b0, min(width, total - b0)) for b0 in range(0, total, width)]


def _chunks(total, maxc):
    n = -(-total // maxc)
    base, rem = divmod(total, n)
    out = []
    off = 0
    for j in range(n):
        ck = base + (1 if j < rem else 0)
        out.append((off, ck))
        off += ck
    return out


def build(C):
    """Build the SPMD single-core program for routed capacity C (mult of 64)."""
    nc = bacc.Bacc("TRN2", target_bir_lowering=False, debug=False, num_devices=N_CORES)

    TOKT = -(-C // P)          # routed token tiles (last may be partial)
    Cp = TOKT * P              # w vector padded to full tiles
    ttiles = [(t * P, min(P, C - t * P)) for t in range(TOKT)]
    tokt_s = CS // P
    cchunks = _chunks(C, 512)

    xg_d = nc.dram_tensor("xg", [H, C], MM_DT, kind="ExternalInput").ap()
    w_d = nc.dram_tensor("w", [P, C], F32, kind="ExternalInput").ap()
    wg_d = nc.dram_tensor("wg", [H, I], MM_DT, kind="ExternalInput").ap()
    wu_d = nc.dram_tensor("wu", [H, I], MM_DT, kind="ExternalInput").ap()
    wd_d = nc.dram_tensor("wd", [I, H], MM_DT, kind="ExternalInput").ap()
    xs_d = nc.dram_tensor("xs", [H, CS], MM_DT, kind="ExternalInput").ap()
    wgs_d = nc.dram_tensor("wgs", [H, I], MM_DT, kind="ExternalInput").ap()
    wus_d = nc.dram_tensor("wus", [H, I], MM_DT, kind="ExternalInput").ap()
    wds_d = nc.dram_tensor("wds", [I, H], MM_DT, kind="ExternalInput").ap()
    yr_d = nc.dram_tensor("yr", [H, C], F32, kind="ExternalOutput").ap()
    ys_d = nc.dram_tensor("ys", [CS, H], F32, kind="ExternalOutput").ap()

    with tile.TileContext(nc) as tc:
        with (
            tc.tile_pool(name="wbig", bufs=4 * HK) as wbig,
            tc.tile_pool(name="wshp", bufs=2 * HK) as wshp,
            tc.tile_pool(name="wdp", bufs=IT) as wdp,
            tc.tile_pool(name="wdsp", bufs=IT) as wdsp,
            tc.tile_pool(name="xp", bufs=HK) as xp,
            tc.tile_pool(name="xsp", bufs=HK) as xsp,
            tc.tile_pool(name="atp", bufs=IT) as atp,
            tc.tile_pool(name="atsp", bufs=IT) as atsp,
            tc.tile_pool(name="gtp", bufs=6) as gtp,
            tc.tile_pool(name="op", bufs=6) as op,
            tc.tile_pool(name="wsp", bufs=1) as wsp,
            tc.tile_pool(name="gps", bufs=4, space="PSUM") as gps,
            tc.tile_pool(name="yps", bufs=4, space="PSUM") as yps,
        ):
            nm = iter(range(10**6))

            # ---- loads for the routed expert (in first-needed order) ----
            xg_sb = []
            for k in range(HK):
                t = xp.tile([P, C], MM_DT, tag="xg", name=f"xg{k}")
                nc.scalar.dma_start(t[:], xg_d[k * P:(k + 1) * P, :])
                xg_sb.append(t)

            # wg/wu streamed in 2 column halves, dual-queue issue: wg on the
            # sync queue, wu on the scalar queue (both HWDGE-capable) so the
            # ~600ns/DMA serial issue cost is paid in parallel. The scalar
            # queue gets ONLY these 8 early DMAs - its silu work starts
            # after they are issued, so nothing blocks.
            wblocks = [(0, 768), (768, I - 768)]
            wg_sb = [[None] * len(wblocks) for _ in range(HK)]
            wu_sb = [[None] * len(wblocks) for _ in range(HK)]
            for b, (b0, bw) in enumerate(wblocks):
                for lst, src, eng in (
                    (wg_sb, wg_d, nc.sync),
                    (wu_sb, wu_d, nc.sync),
                ):
                    for k in range(HK):
                        t = wbig.tile([P, bw], MM_DT, tag="wb", name=f"wb{next(nm)}")
                        eng.dma_start(t[:], src[k * P:(k + 1) * P, b0:b0 + bw])
                        lst[k][b] = t

            def wslice(lst, k, i):
                c = i * P
                b = 0 if c < wblocks[0][1] else 1
                off = c - wblocks[b][0]
                return lst[k][b][:, off:off + P]

            wd_sb = []
            for i in range(IT):
                t = wdp.tile([P, H], MM_DT, tag="wd", name=f"wd{i}")
                nc.sync.dma_start(t[:], wd_d[i * P:(i + 1) * P, :])
                wd_sb.append(t)

            # w is not needed until routed phase 2 - load late so it does
            # not steal early HBM bandwidth from the phase-1 weight stream
            w_sb = wsp.tile([P, C], F32, tag="w")
            nc.sync.dma_start(w_sb[:], w_d[:])

            # ---- loads for the shared expert (scalar queue) ----
            xs_sb = []
            for k in range(HK):
                t = xsp.tile([P, CS], MM_DT, tag="xs", name=f"xs{k}")
                nc.sync.dma_start(t[:], xs_d[k * P:(k + 1) * P, :])
                xs_sb.append(t)
            wgs_sb, wus_sb = [], []
            for wlist, src in ((wgs_sb, wgs_d), (wus_sb, wus_d)):
                for k in range(HK):
                    t = wshp.tile([P, I], MM_DT, tag="wsh", name=f"wb{next(nm)}")
                    nc.sync.dma_start(t[:], src[k * P:(k + 1) * P, :])
                    wlist.append(t)
            wds_sb = []
            for i in range(IT):
                t = wdsp.tile([P, H], MM_DT, tag="wds", name=f"wds{i}")
                nc.sync.dma_start(t[:], wds_d[i * P:(i + 1) * P, :])
                wds_sb.append(t)

            # ---- routed phase 1: AT[i] = silu(Wg^T x) * (Wu^T x) ----
            at_sb = []
            for i in range(IT):
                at_i = atp.tile([P, C], MM_DT, tag="at", name=f"at{i}")
                for (c0, ck) in cchunks:
                    pg = gps.tile([P, ck], F32, tag="pg", name=f"pg{next(nm)}")
                    pu = gps.tile([P, ck], F32, tag="pg", name=f"pu{next(nm)}")
                    for k in range(HK):
                        nc.tensor.matmul(
                            pg[:], wslice(wg_sb, k, i), xg_sb[k][:, c0:c0 + ck],
                            start=(k == 0), stop=(k == HK - 1),
                        )
                    for k in range(HK):
                        nc.tensor.matmul(
                            pu[:], wslice(wu_sb, k, i), xg_sb[k][:, c0:c0 + ck],
                            start=(k == 0), stop=(k == HK - 1),
                        )
                    gt = gtp.tile([P, ck], F32, tag="gt", name=f"gt{next(nm)}")
                    nc.scalar.activation(
                        gt[:], pg[:], mybir.ActivationFunctionType.Silu
                    )
                    nc.vector.tensor_mul(at_i[:, c0:c0 + ck], gt[:], pu[:])
                at_sb.append(at_i)

            # ---- routed phase 2 (transposed): YT[h,:] = sum_i Wd[i][:,h].T @ AT[i],
            # then scale along the token (free) axis by w. Cost scales with C
            # itself rather than ceil(C/128) PSUM sweeps of 512 rows. ----
            for hpair in range(0, HK, 2):
                pyt = {}
                for h in range(hpair, hpair + 2):
                    for ci, (c0, ck) in enumerate(cchunks):
                        pyt[h, ci] = yps.tile([P, max(c[1] for c in cchunks)], F32,
                                              tag="py", name=f"pyt{h}_{ci}")
                for i in range(IT):
                    for h in range(hpair, hpair + 2):
                        for ci, (c0, ck) in enumerate(cchunks):
                            nc.tensor.matmul(
                                pyt[h, ci][:, :ck],
                                wd_sb[i][:, h * P:(h + 1) * P],
                                at_sb[i][:, c0:c0 + ck],
                                start=(i == 0), stop=(i == IT - 1),
                            )
                for h in range(hpair, hpair + 2):
                    o = op.tile([P, C], F32, tag="ot", name=f"o{next(nm)}")
                    for ci, (c0, ck) in enumerate(cchunks):
                        nc.vector.tensor_mul(
                            o[:, c0:c0 + ck], pyt[h, ci][:, :ck],
                            w_sb[:, c0:c0 + ck],
                        )
                    nc.sync.dma_start(yr_d[h * P:(h + 1) * P, :], o[:])

            # ---- shared phase 1 ----
            ats_sb = []
            for i in range(IT):
                at_i = atsp.tile([P, CS], MM_DT, tag="ats", name=f"ats{i}")
                pg = gps.tile([P, CS], F32, tag="pg", name=f"pg{next(nm)}")
                pu = gps.tile([P, CS], F32, tag="pg", name=f"pu{next(nm)}")
                for k in range(HK):
                    nc.tensor.matmul(
                        pg[:], wgs_sb[k][:, i * P:(i + 1) * P], xs_sb[k][:],
                        start=(k == 0), stop=(k == HK - 1),
                    )
                for k in range(HK):
                    nc.tensor.matmul(
                        pu[:], wus_sb[k][:, i * P:(i + 1) * P], xs_sb[k][:],
                        start=(k == 0), stop=(k == HK - 1),
                    )
                gt = gtp.tile([P, CS], F32, tag="gt", name=f"gt{next(nm)}")
                nc.scalar.activation(gt[:], pg[:], mybir.ActivationFunctionType.Silu)
                nc.vector.tensor_mul(at_i[:], gt[:], pu[:])
                ats_sb.append(at_i)

            # ---- shared phase 2 ----
            for t in range(tokt_s):
                py = yps.tile([P, H], F32, tag="py", name=f"py{next(nm)}")
                for i in range(IT):
                    nc.tensor.matmul(
                        py[:], ats_sb[i][:, t * P:(t + 1) * P], wds_sb[i][:],
                        start=(i == 0), stop=(i == IT - 1),
                    )
                o = op.tile([P, H], F32, tag="o", name=f"o{next(nm)}")
                for half in range(2):
                    hs = slice(half * (H // 2), (half + 1) * (H // 2))
                    if half == 0:
                        nc.scalar.activation(
                            o[:, hs], py[:, hs],
                            mybir.ActivationFunctionType.Copy,
                        )
                    else:
                        nc.vector.tensor_copy(o[:, hs], py[:, hs])
                    nc.sync.dma_start(ys_d[t * P:(t + 1) * P, hs], o[:, hs])

    nc.compile()
    return nc


def _gate(x, gate_weight):
    """Host gate: softmax scores, top-2 (lax.top_k tie semantics), combine
    weights, aux loss. Mirrors the reference math in float32."""
    flat = np.ascontiguousarray(x.reshape(T, H), dtype=np.float32)
    logits = flat @ gate_weight.T.astype(np.float32)
    m = logits.max(-1, keepdims=True)
    ex = np.exp(logits - m, dtype=np.float32)
    scores = ex / ex.sum(-1, keepdims=True)
    order = np.argsort(-scores, axis=-1, kind="stable")
    topk_idx = order[:, :K]
    topk_w = np.take_along_axis(scores, topk_idx, axis=-1)
    topk_w = topk_w / (topk_w.sum(-1, keepdims=True) + np.float32(1e-20))

    onehot = np.zeros((T, K, E), np.float32)
    np.put_along_axis(onehot, topk_idx[:, :, None], np.float32(1.0), axis=2)
    counts = onehot.reshape(B, S * K, E).sum(1) / (S * K / E)
    aux = (counts * scores.reshape(B, S, E).mean(1)).sum(-1).mean() * ALPHA

    return flat, scores, topk_idx, topk_w, np.float32(aux)


def kernel(x, gate_weight, Wg, Wu, Wd, Wg_s, Wu_s, Wd_s):
    _install_axon_ntff_shim()
    x = np.asarray(x, np.float32)
    gate_weight = np.asarray(gate_weight, np.float32)
    from concourse.bass_utils import run_bass_kernel_spmd

    flat, scores, topk_idx, topk_w, aux = _gate(x, gate_weight)

    # dispatch: token lists + combine weights per expert
    idx_e, w_e = [], []
    for e in range(E):
        hit = topk_idx == e  # [T, K]
        idx = np.nonzero(hit.any(-1))[0]
        wv = (topk_w * hit).sum(-1)[idx].astype(np.float32)
        idx_e.append(idx)
        w_e.append(wv)
    C = max(512, -(-max(len(i) for i in idx_e) // 32) * 32)

    Wg = np.asarray(Wg, np.float32)
    Wu = np.asarray(Wu, np.float32)
    Wd = np.asarray(Wd, np.float32)
    in_maps = []
    for e in range(E):
        n = len(idx_e[e])
        xg = np.zeros((H, C), np.float32)
        xg[:, :n] = flat[idx_e[e]].T
        w = np.zeros((C,), np.float32)
        w[:n] = w_e[e]
        w = np.broadcast_to(w, (P, C)).copy()
        xs = flat[e * CS:(e + 1) * CS].T
        in_maps.append({
            "xg": _to_mm_np(xg),
            "w": w,
            "wg": _to_mm_np(Wg[e]),
            "wu": _to_mm_np(Wu[e]),
            "wd": _to_mm_np(Wd[e]),
            "xs": _to_mm_np(xs),
            "wgs": _to_mm_np(Wg_s),
            "wus": _to_mm_np(Wu_s),
            "wds": _to_mm_np(Wd_s),
        })

    nc = build(C)
    res = run_bass_kernel_spmd(nc, in_maps, core_ids=list(range(N_CORES)))
    kernel.last_results = res

    # combine: scatter-add routed partials, concat shared partials
    out = np.zeros((T, H), np.float32)
    for e in range(E):
        n = len(idx_e[e])
        out[idx_e[e]] += res.results[e]["yr"][:, :n].T
        out[e * CS:(e + 1) * CS] += res.results[e]["ys"]

    return out.reshape(B, S, H), aux


# revision 35
# speedup vs baseline: 1.1914x; 1.0633x over previous
"""MiniMind MoE FFN on 8 Trainium2 NeuronCores (Bass/Tile, SPMD).

Sharding: expert-parallel. Core e owns routed expert e (Wg/Wu/Wd[e]); tokens
routed to e are gathered host-side (dispatch) and sent as that core's
activation shard, transposed to [H, C] for the matmul layout. The shared
expert is data-parallel over tokens (T/8 per core). Combine (scatter-add of
weighted expert outputs + shared partials) happens at unshard time.

Device math per core, matmul operands in float16 (1 cycle/row on the PE,
half the HBM traffic of fp32; fp32 PSUM accumulation keeps rel-err ~5e-4):
  phase 1:  GT[i] = Wg^T x^T, UT[i] = Wu^T x^T   (I on partitions)
            AT[i] = silu(GT[i]) * UT[i]
  phase 2:  YT[h] = sum_i Wd[i][:,h].T @ AT[i]   (transposed: H on
            partitions, tokens on the free axis, so cost scales with the
            true capacity C rather than ceil(C/128) PSUM sweeps)
            yr    = YT * w[broadcast]            (top-k combine weight)
plus the same for the shared expert on this core's token slice (no w,
token-major output). No PE warmup is needed: the framework preamble's
per-engine TENSOR_LOADs already lift the HAM clock gate.
"""

import sys

sys.path.insert(0, "/opt/trn_rl_repo")

import numpy as np

import concourse.bacc as bacc
import concourse.mybir as mybir
import concourse.tile as tile

H = 512
I = 1408
E = 8
K = 2
ALPHA = 0.1
B, S = 2, 1024
T = B * S
N_CORES = 8
P = 128
HK = H // P     # 4 contraction chunks over H
IT = I // P     # 11 chunks over I
CS = T // N_CORES  # shared-expert tokens per core

F32 = mybir.dt.float32
F32R = mybir.dt.float32r

# matmul-operand dtype: float16 halves DMA vs f32 and runs 1 cycle/row on PE
# (vs ~2cyc/row + 4x weight-load cost for fp32); 10 mantissa bits keep the
# end-to-end error ~1e-3 for these O(1) activations.
import os as _os

_DT_NAME = _os.environ.get("KDT", "f16")
MM_DT = {"f32r": F32R, "f16": mybir.dt.float16, "bf16": mybir.dt.bfloat16}[_DT_NAME]


def _to_mm_np(a):
    import ml_dtypes

    np_dt = {"f32r": np.float32, "f16": np.float16, "bf16": ml_dtypes.bfloat16}[
        _DT_NAME
    ]
    return np.ascontiguousarray(np.asarray(a, np.float32).astype(np_dt))


def _install_axon_ntff_shim():
    """Provide antenv.axon_hooks if the image lacks it, so that
    run_bass_kernel_spmd(trace=True)/BASS_TRACE doesn't crash under axon."""
    try:
        import antenv.axon_hooks  # noqa: F401

        return
    except ImportError:
        pass
    import contextlib
    import ctypes
    import types

    def _make_hook():
        try:
            lib = ctypes.CDLL("/opt/axon/libaxon_pjrt.so")
            if not hasattr(lib, "axon_start_nrt_profile"):
                return None
            lib.axon_start_nrt_profile.argtypes = [
                ctypes.POINTER(ctypes.c_int64),
                ctypes.c_size_t,
            ]
            lib.axon_start_nrt_profile.restype = ctypes.c_int64
            lib.axon_stop_nrt_profile.argtypes = [ctypes.c_char_p]
            lib.axon_stop_nrt_profile.restype = ctypes.c_int64
        except OSError:
            return None

        @contextlib.contextmanager
        def _hook(output_dir, device_ids):
            import jax

            jax.devices()
            if device_ids:
                ids = (ctypes.c_int64 * len(device_ids))(*device_ids)
                rc = lib.axon_start_nrt_profile(ids, len(device_ids))
            else:
                rc = lib.axon_start_nrt_profile(None, 0)
            if rc != 0:
                raise RuntimeError(f"axon_start_nrt_profile rc={rc}")
            try:
                yield
            finally:
                n = lib.axon_stop_nrt_profile(str(output_dir).encode())
                print(f"ntff profile: {n} file(s) in {output_dir}", file=sys.stderr)

        return _hook

    hook = _make_hook()
    mod = types.ModuleType("antenv.axon_hooks")
    mod.get_axon_ntff_profile_hook = lambda: hook
    mod.set_axon_ntff_profile_hook = lambda h: None
    sys.modules["antenv.axon_hooks"] = mod


def _blocks(total, width):
    return [(b0, min(width, total - b0)) for b0 in range(0, total, width)]


def _chunks(total, maxc):
    n = -(-total // maxc)
    base, rem = divmod(total, n)
    out = []
    off = 0
    for j in range(n):
        ck = base + (1 if j < rem else 0)
        out.append((off, ck))
        off += ck
    return out


def build(C):
    """Build the SPMD single-core program for routed capacity C (mult of 64)."""
    nc = bacc.Bacc("TRN2", target_bir_lowering=False, debug=False, num_devices=N_CORES)

    TOKT = -(-C // P)          # routed token tiles (last may be partial)
    Cp = TOKT * P              # w vector padded to full tiles
    ttiles = [(t * P, min(P, C - t * P)) for t in range(TOKT)]
    tokt_s = CS // P
    cchunks = _chunks(C, 512)

    xg_d = nc.dram_tensor("xg", [H, C], MM_DT, kind="ExternalInput").ap()
    w_d = nc.dram_tensor("w", [P, C], F32, kind="ExternalInput").ap()
    wg_d = nc.dram_tensor("wg", [H, I], MM_DT, kind="ExternalInput").ap()
    wu_d = nc.dram_tensor("wu", [H, I], MM_DT, kind="ExternalInput").ap()
    wd_d = nc.dram_tensor("wd", [I, H], MM_DT, kind="ExternalInput").ap()
    xs_d = nc.dram_tensor("xs", [H, CS], MM_DT, kind="ExternalInput").ap()
    wgs_d = nc.dram_tensor("wgs", [H, I], MM_DT, kind="ExternalInput").ap()
    wus_d = nc.dram_tensor("wus", [H, I], MM_DT, kind="ExternalInput").ap()
    wds_d = nc.dram_tensor("wds", [I, H], MM_DT, kind="ExternalInput").ap()
    yr_d = nc.dram_tensor("yr", [H, C], F32, kind="ExternalOutput").ap()
    ys_d = nc.dram_tensor("ys", [CS, H], F32, kind="ExternalOutput").ap()

    with tile.TileContext(nc) as tc:
        with (
            tc.tile_pool(name="wbig", bufs=4 * HK) as wbig,
            tc.tile_pool(name="wshp", bufs=2 * HK) as wshp,
            tc.tile_pool(name="wdp", bufs=IT) as wdp,
            tc.tile_pool(name="wdsp", bufs=IT) as wdsp,
            tc.tile_pool(name="xp", bufs=HK) as xp,
            tc.tile_pool(name="xsp", bufs=HK) as xsp,
            tc.tile_pool(name="atp", bufs=IT) as atp,
            tc.tile_pool(name="atsp", bufs=IT) as atsp,
            tc.tile_pool(name="gtp", bufs=3) as gtp,
            tc.tile_pool(name="op", bufs=4) as op,
            tc.tile_pool(name="wsp", bufs=1) as wsp,
            tc.tile_pool(name="gps", bufs=4, space="PSUM") as gps,
            tc.tile_pool(name="yps", bufs=4, space="PSUM") as yps,
        ):
            nm = iter(range(10**6))

            # ---- loads for the routed expert (in first-needed order) ----
            xg_sb = []
            for k in range(HK):
                t = xp.tile([P, C], MM_DT, tag="xg", name=f"xg{k}")
                nc.scalar.dma_start(t[:], xg_d[k * P:(k + 1) * P, :])
                xg_sb.append(t)

            # wg/wu streamed in 2 column halves, dual-queue issue: wg on the
            # sync queue, wu on the scalar queue (both HWDGE-capable) so the
            # ~600ns/DMA serial issue cost is paid in parallel. The scalar
            # queue gets ONLY these 8 early DMAs - its silu work starts
            # after they are issued, so nothing blocks.
            wblocks = [(0, 768), (768, I - 768)]
            wg_sb = [[None] * len(wblocks) for _ in range(HK)]
            wu_sb = [[None] * len(wblocks) for _ in range(HK)]
            for b, (b0, bw) in enumerate(wblocks):
                for lst, src, eng in (
                    (wg_sb, wg_d, nc.sync),
                    (wu_sb, wu_d, nc.sync),
                ):
                    for k in range(HK):
                        t = wbig.tile([P, bw], MM_DT, tag="wb", name=f"wb{next(nm)}")
                        eng.dma_start(t[:], src[k * P:(k + 1) * P, b0:b0 + bw])
                        lst[k][b] = t

            def wslice(lst, k, i):
                c = i * P
                b = 0 if c < wblocks[0][1] else 1
                off = c - wblocks[b][0]
                return lst[k][b][:, off:off + P]

            wd_sb = []
            for i in range(IT):
                t = wdp.tile([P, H], MM_DT, tag="wd", name=f"wd{i}")
                nc.sync.dma_start(t[:], wd_d[i * P:(i + 1) * P, :])
                wd_sb.append(t)

            # w is not needed until routed phase 2 - load late so it does
            # not steal early HBM bandwidth from the phase-1 weight stream
            w_sb = wsp.tile([P, C], F32, tag="w")
            nc.sync.dma_start(w_sb[:], w_d[:])

            # ---- loads for the shared expert (scalar queue) ----
            xs_sb = []
            for k in range(HK):
                t = xsp.tile([P, CS], MM_DT, tag="xs", name=f"xs{k}")
                nc.sync.dma_start(t[:], xs_d[k * P:(k + 1) * P, :])
                xs_sb.append(t)
            wgs_sb, wus_sb = [], []
            for wlist, src in ((wgs_sb, wgs_d), (wus_sb, wus_d)):
                for k in range(HK):
                    t = wshp.tile([P, I], MM_DT, tag="wsh", name=f"wb{next(nm)}")
                    nc.sync.dma_start(t[:], src[k * P:(k + 1) * P, :])
                    wlist.append(t)
            wds_sb = []
            for i in range(IT):
                t = wdsp.tile([P, H], MM_DT, tag="wds", name=f"wds{i}")
                nc.sync.dma_start(t[:], wds_d[i * P:(i + 1) * P, :])
                wds_sb.append(t)

            # ---- routed phase 1: AT[i] = silu(Wg^T x) * (Wu^T x) ----
            at_sb = []
            for i in range(IT):
                at_i = atp.tile([P, C], MM_DT, tag="at", name=f"at{i}")
                for (c0, ck) in cchunks:
                    pg = gps.tile([P, ck], F32, tag="pg", name=f"pg{next(nm)}")
                    pu = gps.tile([P, ck], F32, tag="pg", name=f"pu{next(nm)}")
                    for k in range(HK):
                        nc.tensor.matmul(
                            pg[:], wslice(wg_sb, k, i), xg_sb[k][:, c0:c0 + ck],
                            start=(k == 0), stop=(k == HK - 1),
                        )
                    for k in range(HK):
                        nc.tensor.matmul(
                            pu[:], wslice(wu_sb, k, i), xg_sb[k][:, c0:c0 + ck],
                            start=(k == 0), stop=(k == HK - 1),
                        )
                    gt = gtp.tile([P, ck], F32, tag="gt", name=f"gt{next(nm)}")
                    nc.scalar.activation(
                        gt[:], pg[:], mybir.ActivationFunctionType.Silu
                    )
                    nc.vector.tensor_mul(at_i[:, c0:c0 + ck], gt[:], pu[:])
                at_sb.append(at_i)

            # ---- routed phase 2 (transposed): YT[h,:] = sum_i Wd[i][:,h].T @ AT[i],
            # then scale along the token (free) axis by w. Cost scales with C
            # itself rather than ceil(C/128) PSUM sweeps of 512 rows. ----
            for hpair in range(0, HK, 2):
                pyt = {}
                for h in range(hpair, hpair + 2):
                    for ci, (c0, ck) in enumerate(cchunks):
                        pyt[h, ci] = yps.tile([P, max(c[1] for c in cchunks)], F32,
                                              tag="py", name=f"pyt{h}_{ci}")
                for i in range(IT):
                    for h in range(hpair, hpair + 2):
                        for ci, (c0, ck) in enumerate(cchunks):
                            nc.tensor.matmul(
                                pyt[h, ci][:, :ck],
                                wd_sb[i][:, h * P:(h + 1) * P],
                                at_sb[i][:, c0:c0 + ck],
                                start=(i == 0), stop=(i == IT - 1),
                            )
                for h in range(hpair, hpair + 2):
                    o = op.tile([P, C], F32, tag="ot", name=f"o{next(nm)}")
                    for ci, (c0, ck) in enumerate(cchunks):
                        nc.vector.tensor_mul(
                            o[:, c0:c0 + ck], pyt[h, ci][:, :ck],
                            w_sb[:, c0:c0 + ck],
                        )
                    nc.sync.dma_start(yr_d[h * P:(h + 1) * P, :], o[:])

            # ---- shared phase 1 ----
            ats_sb = []
            for i in range(IT):
                at_i = atsp.tile([P, CS], MM_DT, tag="ats", name=f"ats{i}")
                pg = gps.tile([P, CS], F32, tag="pg", name=f"pg{next(nm)}")
                pu = gps.tile([P, CS], F32, tag="pg", name=f"pu{next(nm)}")
                for k in range(HK):
                    nc.tensor.matmul(
                        pg[:], wgs_sb[k][:, i * P:(i + 1) * P], xs_sb[k][:],
                        start=(k == 0), stop=(k == HK - 1),
                    )
                for k in range(HK):
                    nc.tensor.matmul(
                        pu[:], wus_sb[k][:, i * P:(i + 1) * P], xs_sb[k][:],
                        start=(k == 0), stop=(k == HK - 1),
                    )
                gt = gtp.tile([P, CS], F32, tag="gt", name=f"gt{next(nm)}")
                nc.scalar.activation(gt[:], pg[:], mybir.ActivationFunctionType.Silu)
                nc.vector.tensor_mul(at_i[:], gt[:], pu[:])
                ats_sb.append(at_i)

            # ---- shared phase 2 ----
            for t in range(tokt_s):
                py = yps.tile([P, H], F32, tag="py", name=f"py{next(nm)}")
                for i in range(IT):
                    nc.tensor.matmul(
                        py[:], ats_sb[i][:, t * P:(t + 1) * P], wds_sb[i][:],
                        start=(i == 0), stop=(i == IT - 1),
                    )
                o = op.tile([P, H], F32, tag="o", name=f"o{next(nm)}")
                for half in range(2):
                    hs = slice(half * (H // 2), (half + 1) * (H // 2))
                    if half == 0:
                        nc.scalar.activation(
                            o[:, hs], py[:, hs],
                            mybir.ActivationFunctionType.Copy,
                        )
                    else:
                        nc.vector.tensor_copy(o[:, hs], py[:, hs])
                    nc.sync.dma_start(ys_d[t * P:(t + 1) * P, hs], o[:, hs])

    nc.compile()
    return nc


def _gate(x, gate_weight):
    """Host gate: softmax scores, top-2 (lax.top_k tie semantics), combine
    weights, aux loss. Mirrors the reference math in float32."""
    flat = np.ascontiguousarray(x.reshape(T, H), dtype=np.float32)
    logits = flat @ gate_weight.T.astype(np.float32)
    m = logits.max(-1, keepdims=True)
    ex = np.exp(logits - m, dtype=np.float32)
    scores = ex / ex.sum(-1, keepdims=True)
    order = np.argsort(-scores, axis=-1, kind="stable")
    topk_idx = order[:, :K]
    topk_w = np.take_along_axis(scores, topk_idx, axis=-1)
    topk_w = topk_w / (topk_w.sum(-1, keepdims=True) + np.float32(1e-20))

    onehot = np.zeros((T, K, E), np.float32)
    np.put_along_axis(onehot, topk_idx[:, :, None], np.float32(1.0), axis=2)
    counts = onehot.reshape(B, S * K, E).sum(1) / (S * K / E)
    aux = (counts * scores.reshape(B, S, E).mean(1)).sum(-1).mean() * ALPHA

    return flat, scores, topk_idx, topk_w, np.float32(aux)


def kernel(x, gate_weight, Wg, Wu, Wd, Wg_s, Wu_s, Wd_s):
    _install_axon_ntff_shim()
    x = np.asarray(x, np.float32)
    gate_weight = np.asarray(gate_weight, np.float32)
    from concourse.bass_utils import run_bass_kernel_spmd

    flat, scores, topk_idx, topk_w, aux = _gate(x, gate_weight)

    # dispatch: token lists + combine weights per expert
    idx_e, w_e = [], []
    for e in range(E):
        hit = topk_idx == e  # [T, K]
        idx = np.nonzero(hit.any(-1))[0]
        wv = (topk_w * hit).sum(-1)[idx].astype(np.float32)
        idx_e.append(idx)
        w_e.append(wv)
    C = max(512, -(-max(len(i) for i in idx_e) // 32) * 32)

    Wg = np.asarray(Wg, np.float32)
    Wu = np.asarray(Wu, np.float32)
    Wd = np.asarray(Wd, np.float32)
    in_maps = []
    for e in range(E):
        n = len(idx_e[e])
        xg = np.zeros((H, C), np.float32)
        xg[:, :n] = flat[idx_e[e]].T
        w = np.zeros((C,), np.float32)
        w[:n] = w_e[e]
        w = np.broadcast_to(w, (P, C)).copy()
        xs = flat[e * CS:(e + 1) * CS].T
        in_maps.append({
            "xg": _to_mm_np(xg),
            "w": w,
            "wg": _to_mm_np(Wg[e]),
            "wu": _to_mm_np(Wu[e]),
            "wd": _to_mm_np(Wd[e]),
            "xs": _to_mm_np(xs),
            "wgs": _to_mm_np(Wg_s),
            "wus": _to_mm_np(Wu_s),
            "wds": _to_mm_np(Wd_s),
        })

    nc = build(C)
    res = run_bass_kernel_spmd(nc, in_maps, core_ids=list(range(N_CORES)))
    kernel.last_results = res

    # combine: scatter-add routed partials, concat shared partials
    out = np.zeros((T, H), np.float32)
    for e in range(E):
        n = len(idx_e[e])
        out[idx_e[e]] += res.results[e]["yr"][:, :n].T
        out[e * CS:(e + 1) * CS] += res.results[e]["ys"]

    return out.reshape(B, S, H), aux
